# revision 15
# baseline (speedup 1.0000x reference)
"""GCN-VAE encoder (2-layer GCN + reparameterize) on 8 Trainium2 NeuronCores.

Strategy (per the dst-sharding hint):
  - Nodes are sharded across the 8 cores (6250 rows each); edges are
    partitioned by dst and sorted by dst within each core.
  - Layer matmuls (x@W1, h1@[W2|W3]) run on the node shard each core owns.
  - The sparse aggregation out[d] = sum_{(s,d) in E} w * feat[s] is computed
    per 128-dst-node "window": edges are chunked into groups of 128, features
    feat[src] are fetched with dma_gather row gathers (two per window — the
    int16 gather index forces a <32768 split of the feature table), and the
    segment-sum is a PE matmul acc += S^T @ G with a one-hot matrix
    S[e, dst_local[e]] = w_e built on the vector engine.
  - Cross-core exchange of the full feature tables (support1, support23)
    happens through host round-trips between three SPMD launches (no
    on-device collectives needed):
      L1: support1_shard = x_shard @ W1
      L2: h1 = relu(spmm(support1_full));  support23_shard = h1 @ [W2|W3]
      L3: [mu|logvar] = relu(spmm(support23_full)); z = eps*exp(logvar)+mu
"""

import sys

for _p in ("/opt/trn_rl_repo", "/root/.axon_site/_ro/trn_rl_repo"):
    if _p not in sys.path:
        sys.path.append(_p)

import numpy as np
import ml_dtypes

import concourse.bass as bass
import concourse.mybir as mybir
import concourse.tile as tile
from concourse import bacc
from concourse.bass_utils import run_bass_kernel_spmd
from concourse.masks import make_identity

# ---- problem constants (hardcoded per harness contract) ----
N, E, F_IN, H1, H2 = 50000, 1600000, 512, 256, 64
H23 = 2 * H2                      # concat(mu, logvar) feature width
M = 8                             # cores
NSH = N // M                      # nodes per core
P = 128                           # partitions / window size / edge chunk
NWIN = (NSH + P - 1) // P         # dst windows per core (49)
KCH = F_IN // P                   # k-chunks for layer-1 matmul (4)
SPLIT = 32768                     # int16 gather-index limit

f32 = mybir.dt.float32
i16 = mybir.dt.int16

DT = {"f32": mybir.dt.float32, "f32r": mybir.dt.float32r,
      "f16": mybir.dt.float16, "bf16": mybir.dt.bfloat16}
NPDT = {"f32": np.float32, "f32r": np.float32,
        "f16": np.float16, "bf16": ml_dtypes.bfloat16}

# dtype ladder (accuracy/speed): "f32" exact, "f32r" single-pass PE fp32,
# "f16"/"bf16" half-width gather tables.
L1_DT = "f16"                     # x/W1 matmul operand dtype
SUP1_DT = "f16"                  # layer-1 feature table + S dtype
SUP23_DT = "f16"                  # layer-2/3 feature table + S dtype

_PROG_CACHE: dict = {}


# ---------------------------------------------------------------- host prep
def _wrap16(arr):
    """int16 gather-index layout: ordinal i -> [i%16, i//16], replicated to
    128 partitions (8 Q7 cores x 16)."""
    w = arr.astype(np.int16).reshape(-1, 16).T
    return np.tile(w, (8, 1))


def _prep_edges(edge_src, edge_dst, edge_weight):
    """Partition edges by dst shard, sort by (window, src-half), lay out
    per-window chunk tiles.

    Device arrays per core:
      EIDX [128, (totA+totB)*8] int16 — per window [A idxs | B idxs] wrapped
      EMETA [128, 2*totc] f32 — per window [dst_local (nw) | weight (nw)]
    Returns (key, meta, shards); meta holds the static chunk structure
    (identical across cores by construction)."""
    edge_src = np.asarray(edge_src).astype(np.int64)
    edge_dst = np.asarray(edge_dst).astype(np.int64)
    edge_weight = np.asarray(edge_weight).astype(np.float32)

    percore = []
    cntA = np.zeros((M, NWIN), np.int64)
    cntB = np.zeros((M, NWIN), np.int64)
    perms = []
    for m in range(M):
        sel = (edge_dst >= m * NSH) & (edge_dst < (m + 1) * NSH)
        d = edge_dst[sel] - m * NSH
        s = edge_src[sel]
        w = edge_weight[sel]
        win0 = d >> 7
        # slot-balance: rank windows by edge count so slot j holds each
        # core's j-th-largest window -> cross-core max padding shrinks
        wcnt = np.bincount(win0, minlength=NWIN)
        perm = np.argsort(-wcnt, kind="stable")           # slot -> window
        inv = np.empty(NWIN, np.int64)
        inv[perm] = np.arange(NWIN)                       # window -> slot
        perms.append(perm)
        win = inv[win0]                                   # slot index per edge
        klass = np.where(s < N - SPLIT, 0,
                         np.where(s < SPLIT, 1, 2))       # A-only/flex/B-only
        order = np.lexsort((klass, win))
        d, s, w, win, klass = d[order], s[order], w[order], win[order], klass[order]
        dloc = d - (perm[win] << 7)                       # dst_local in window
        for h, cnt in ((0, cntA), (2, cntB)):
            msk = klass == h
            cnt[m] = np.bincount(win[msk], minlength=NWIN)
        percore.append((dloc, s, w, win, klass))

    # chunk budget per slot: total rounded up, A sized to forced-A max,
    # flex edges fill A chunks to capacity before B
    tots = np.stack([np.bincount(pc[3], minlength=NWIN) for pc in percore])
    ncwT = np.maximum(2, -(-tots.max(axis=0) // P))
    ncwA = np.maximum(1, -(-cntA.max(axis=0) // P))
    while True:
        bad = (cntB.max(axis=0) > (ncwT - ncwA) * P)
        if not bad.any():
            break
        ncwT[bad] += 1
    ncwB = ncwT - ncwA
    ncw = ncwT
    offs = np.concatenate([[0], np.cumsum(ncw)])
    offsA = np.concatenate([[0], np.cumsum(ncwA)])
    offsB = np.concatenate([[0], np.cumsum(ncwB)])
    totc = int(offs[-1])
    totA, totB = int(offsA[-1]), int(offsB[-1])

    shards = []
    for m in range(M):
        dloc, s, w, win, klass = percore[m]
        DSTL = np.zeros((P, totc), np.float32)
        WGT = np.zeros((P, totc), np.float32)
        srcA = np.zeros(totA * P, np.int64)
        srcB = np.zeros(totB * P, np.int64)
        # edges are slot-major, class-ordered (A-only, flex, B-only): the
        # first capA go to half A (flex spills into A until its chunks are
        # full), the rest to half B
        wcnt = np.bincount(win, minlength=NWIN)
        starts = np.concatenate([[0], np.cumsum(wcnt)])[:-1]
        jall = np.arange(len(dloc)) - starts[win]          # rank within slot
        capA = ncwA[win] * P
        nAB = np.minimum(wcnt, ncwA * P)                   # A edges per slot
        toA = jall < capA
        for h, offsH, srcH, base_extra, sub in (
                (True, offsA, srcA, 0, 0),
                (False, offsB, srcB, None, SPLIT)):
            msk = toA if h else ~toA
            dh, sh, wh, winh = dloc[msk], s[msk], w[msk], win[msk]
            j = jall[msk] if h else jall[msk] - nAB[winh]
            rows = j % P
            base = offs[winh] + (0 if h else ncwA[winh])
            cols = base + j // P
            DSTL[rows, cols] = dh
            WGT[rows, cols] = wh
            srcH[offsH[winh] * P + j] = sh - (0 if h else N - SPLIT)
        IDXA = _wrap16(srcA)                               # [128, totA*8]
        IDXB = _wrap16(srcB)
        EIDX = np.zeros((P, (totA + totB) * 8), np.int16)
        EMETA = np.zeros((P, 2 * totc), np.float32)
        for t in range(NWIN):
            nA, nB = int(ncwA[t]), int(ncwB[t])
            co = (int(offsA[t]) + int(offsB[t])) * 8
            EIDX[:, co:co + nA * 8] = IDXA[:, offsA[t] * 8:(offsA[t] + nA) * 8]
            EIDX[:, co + nA * 8:co + (nA + nB) * 8] = \
                IDXB[:, offsB[t] * 8:(offsB[t] + nB) * 8]
            o = int(offs[t])
            EMETA[:, 2 * o:2 * o + (nA + nB)] = DSTL[:, o:o + nA + nB]
            EMETA[:, 2 * o + nA + nB:2 * (o + nA + nB)] = WGT[:, o:o + nA + nB]
        shards.append((EIDX, EMETA, perms[m]))

    key = tuple(int(v) for v in ncwA) + tuple(int(v) for v in ncwB)
    meta = (tuple(int(v) for v in ncwA), tuple(int(v) for v in ncwB),
            offs, offsA, offsB, totc, totA, totB)
    return key, meta, shards


# ------------------------------------------------------------- bass builders
def _mk_nc():
    return bacc.Bacc("TRN2", target_bir_lowering=False, debug=False)


def _build_l1():
    """support1_shard[6250,256] = x_shard @ W1.

    xL is host-prepared as [128, NSH, KCH] (xL[p,n,k] = x[n, k*128+p]) so the
    whole shard loads SBUF-resident with four big contiguous DMAs; matmuls
    read stationary tiles straight out of it."""
    dt = DT[L1_DT]
    nc = _mk_nc()
    odt = DT[SUP1_DT]
    xL = nc.dram_tensor("xL", [P, NSH, KCH], dt, kind="ExternalInput")
    W1 = nc.dram_tensor("W1", [F_IN, H1], dt, kind="ExternalInput")
    s1 = nc.dram_tensor("s1", [NSH, H1], odt, kind="ExternalOutput")

    NSPLIT = 8
    spans = [(NSH * i // NSPLIT, NSH * (i + 1) // NSPLIT) for i in range(NSPLIT)]
    with tile.TileContext(nc) as tc:
        with tc.tile_pool(name="const", bufs=1) as cpool, \
             tc.tile_pool(name="sbuf", bufs=6) as pool, \
             tc.tile_pool(name="psum", bufs=4, space="PSUM") as psum:
            w1c = cpool.tile([P, KCH, H1], dt)
            nc.sync.dma_start(out=w1c[:],
                              in_=W1[:].rearrange("(k p) n -> p k n", p=P))
            xfull = cpool.tile([P, NSH, KCH], dt)
            for a, b in spans:
                nc.sync.dma_start(out=xfull[:, a:b, :], in_=xL[:, a:b, :])
            for t in range(NWIN):
                rows = min(P, NSH - t * P)
                acc = psum.tile([P, H1], f32, space="PSUM", tag="acc")
                for k in range(KCH):
                    nc.tensor.matmul(
                        out=acc[:rows, :],
                        lhsT=xfull[:, t * P:t * P + rows, k],
                        rhs=w1c[:, k, :],
                        start=(k == 0), stop=(k == KCH - 1))
                o = pool.tile([P, H1], odt, tag="o")
                nc.scalar.activation(out=o[:rows, :], in_=acc[:rows, :],
                                     func=mybir.ActivationFunctionType.Copy)
                nc.sync.dma_start(out=s1[t * P:t * P + rows, :], in_=o[:rows, :])
    nc.compile()
    return nc


def _spmm_windows(nc, pool, psum, gpool, supA, supB, eidx, emeta, iota_t,
                  meta, H, dt, per_window_out):
    """Shared spmm structure: for each window slot produce PSUM [128, H]
    segment sum, then call per_window_out(win, rows, acc_psum)."""
    ncwA, ncwB, offs, offsA, offsB, totc, totA, totB = meta
    for win in range(NWIN):
        rows = P
        nA, nB = ncwA[win], ncwB[win]
        nw = nA + nB
        off = int(offs[win])
        co = (int(offsA[win]) + int(offsB[win])) * 8
        meta_t = pool.tile([P, 2 * nw], f32, tag="meta")
        nc.sync.dma_start(out=meta_t[:], in_=emeta[:, 2 * off:2 * (off + nw)])
        idx_t = pool.tile([P, nw * 8], i16, tag="idx")
        nc.sync.dma_start(out=idx_t[:], in_=eidx[:, co:co + nw * 8])

        G = gpool.tile([P, nw, H], dt, tag="G")
        nc.gpsimd.dma_gather(
            out_ap=G[:, 0:nA, :], in_ap=supA[:], idxs_ap=idx_t[:, :nA * 8],
            num_idxs=nA * P, num_idxs_reg=nA * P, elem_size=H,
            single_packet=False)
        nc.gpsimd.dma_gather(
            out_ap=G[:, nA:nw, :], in_ap=supB[:], idxs_ap=idx_t[:, nA * 8:],
            num_idxs=nB * P, num_idxs_reg=nB * P, elem_size=H,
            single_packet=False)

        acc = psum.tile([P, H], f32, space="PSUM", tag="acc")
        for c in range(nw):
            S = pool.tile([P, P], dt, tag="S")
            nc.vector.tensor_scalar(
                out=S[:], in0=iota_t[:],
                scalar1=meta_t[:, c:c + 1], scalar2=meta_t[:, nw + c:nw + c + 1],
                op0=mybir.AluOpType.is_equal, op1=mybir.AluOpType.mult)
            nc.tensor.matmul(
                out=acc[:],
                lhsT=S[:],
                rhs=G[:, c, :],
                start=(c == 0), stop=(c == nw - 1))
        per_window_out(win, rows, acc)


def _decl_spmm_inputs(nc, meta, H, dt, supname):
    _, _, _, _, _, totc, totA, totB = meta
    supA = nc.dram_tensor(supname + "A", [SPLIT, H], dt, kind="ExternalInput")
    supB = nc.dram_tensor(supname + "B", [SPLIT, H], dt, kind="ExternalInput")
    eidx = nc.dram_tensor("eidx", [P, (totA + totB) * 8], i16, kind="ExternalInput")
    emeta = nc.dram_tensor("emeta", [P, 2 * totc], f32, kind="ExternalInput")
    iota_h = nc.dram_tensor("iota", [P, P], dt, kind="ExternalInput")
    return supA, supB, eidx, emeta, iota_h


def _build_l2(meta):
    """h1 = relu(spmm(support1)); support23_shard = h1 @ W23."""
    dt = DT[SUP1_DT]
    nc = _mk_nc()
    supA, supB, eidx, emeta, iota_h = _decl_spmm_inputs(nc, meta, H1, dt, "sup1")
    W23 = nc.dram_tensor("W23", [H1, H23], f32, kind="ExternalInput")
    odt = DT[SUP23_DT]
    s23 = nc.dram_tensor("s23", [NWIN * P, H23], odt, kind="ExternalOutput")

    with tile.TileContext(nc) as tc:
        with tc.tile_pool(name="const", bufs=1) as cpool, \
             tc.tile_pool(name="sbuf", bufs=3) as pool, \
             tc.tile_pool(name="small", bufs=8) as spool, \
             tc.tile_pool(name="gpool", bufs=4) as gpool, \
             tc.tile_pool(name="psum", bufs=2, space="PSUM") as psum:
            iota_t = cpool.tile([P, P], dt)
            nc.sync.dma_start(out=iota_t[:], in_=iota_h[:])
            ident = cpool.tile([P, P], f32)
            make_identity(nc, ident[:])
            w23c = cpool.tile([P, H1 // P, H23], f32)
            nc.sync.dma_start(out=w23c[:],
                              in_=W23[:].rearrange("(k p) n -> p k n", p=P))

            def finish(win, rows, acc):
                h1 = pool.tile([P, H1], f32, tag="h1")
                nc.scalar.activation(out=h1[:], in_=acc[:],
                                     func=mybir.ActivationFunctionType.Relu)
                ps23 = psum.tile([P, H23], f32, space="PSUM", tag="ps23")
                for fh in range(H1 // P):
                    tp = psum.tile([P, P], f32, space="PSUM", tag="tp")
                    nc.tensor.transpose(out=tp[:], in_=h1[:, fh * P:(fh + 1) * P],
                                        identity=ident[:])
                    tps = pool.tile([P, P], f32, tag="tps")
                    nc.vector.tensor_copy(out=tps[:], in_=tp[:])
                    nc.tensor.matmul(
                        out=ps23[:],
                        lhsT=tps[:],
                        rhs=w23c[:, fh, :],
                        start=(fh == 0), stop=(fh == H1 // P - 1))
                o = pool.tile([P, H23], odt, tag="o")
                nc.scalar.activation(out=o[:rows, :], in_=ps23[:rows, :],
                                     func=mybir.ActivationFunctionType.Copy)
                nc.sync.dma_start(out=s23[win * P:win * P + rows, :],
                                  in_=o[:rows, :])

            _spmm_windows(nc, spool, psum, gpool, supA, supB, eidx, emeta,
                          iota_t, meta, H1, dt, finish)
    nc.compile()
    return nc


def _build_l3(meta):
    """[mu|logvar] = relu(spmm(support23)); z = eps*exp(logvar)+mu."""
    dt = DT[SUP23_DT]
    nc = _mk_nc()
    supA, supB, eidx, emeta, iota_h = _decl_spmm_inputs(nc, meta, H23, dt, "sup23")
    epss = nc.dram_tensor("epss", [NWIN * P, H2], f32, kind="ExternalInput")
    out3 = nc.dram_tensor("out3", [NWIN * P, 3 * H2], f32, kind="ExternalOutput")

    with tile.TileContext(nc) as tc:
        with tc.tile_pool(name="const", bufs=1) as cpool, \
             tc.tile_pool(name="sbuf", bufs=3) as pool, \
             tc.tile_pool(name="small", bufs=8) as spool, \
             tc.tile_pool(name="gpool", bufs=4) as gpool, \
             tc.tile_pool(name="psum", bufs=3, space="PSUM") as psum:
            iota_t = cpool.tile([P, P], dt)
            nc.sync.dma_start(out=iota_t[:], in_=iota_h[:])

            def finish(win, rows, acc):
                o = pool.tile([P, 3 * H2], f32, tag="o3")
                # o = [z | mu | logvar]
                nc.scalar.activation(out=o[:, H2:H23], in_=acc[:, 0:H2],
                                     func=mybir.ActivationFunctionType.Relu)
                nc.scalar.activation(out=o[:, H23:3 * H2], in_=acc[:, H2:H23],
                                     func=mybir.ActivationFunctionType.Relu)
                ex_t = pool.tile([P, H2], f32, tag="ex")
                nc.scalar.activation(out=ex_t[:], in_=o[:, H23:3 * H2],
                                     func=mybir.ActivationFunctionType.Exp)
                ep_t = pool.tile([P, H2], f32, tag="ep")
                nc.sync.dma_start(out=ep_t[:], in_=epss[win * P:(win + 1) * P, :])
                nc.vector.tensor_mul(out=o[:, 0:H2], in0=ex_t[:], in1=ep_t[:])
                nc.vector.tensor_add(out=o[:, 0:H2], in0=o[:, 0:H2],
                                     in1=o[:, H2:H23])
                nc.sync.dma_start(out=out3[win * P:(win + 1) * P, :], in_=o[:])

            _spmm_windows(nc, spool, psum, gpool, supA, supB, eidx, emeta,
                          iota_t, meta, H23, dt, finish)
    nc.compile()
    return nc


def _get_progs(key, meta):
    ck = (key, L1_DT, SUP1_DT, SUP23_DT)
    if ck not in _PROG_CACHE:
        _PROG_CACHE[ck] = (_build_l1(), _build_l2(meta), _build_l3(meta))
    return _PROG_CACHE[ck]


# ------------------------------------------------------------------- kernel
def _run_spmd(nc, in_maps, tries=4):
    """run_bass_kernel_spmd with retries: the shared device pool occasionally
    needs a few minutes to recover a wedged worker."""
    import time
    for attempt in range(tries):
        try:
            return run_bass_kernel_spmd(nc, in_maps, core_ids=list(range(M)))
        except Exception:
            if attempt == tries - 1:
                raise
            time.sleep(90)


def kernel(x, W1, W2, W3, edge_weight, eps, edge_src, edge_dst):
    x = np.asarray(x, np.float32)
    W1 = np.asarray(W1, np.float32)
    W23 = np.concatenate([np.asarray(W2, np.float32),
                          np.asarray(W3, np.float32)], axis=1)
    eps = np.asarray(eps, np.float32)

    key, meta, eshards = _prep_edges(edge_src, edge_dst, edge_weight)
    nc1, nc2, nc3 = _get_progs(key, meta)

    iota = np.broadcast_to(np.arange(P, dtype=np.float32)[None, :], (P, P))

    # ---- L1: support1 shards
    np1 = NPDT[L1_DT]
    in1 = []
    for m in range(M):
        xs = x[m * NSH:(m + 1) * NSH].astype(np1)          # [NSH, 512]
        xLm = np.ascontiguousarray(
            xs.reshape(NSH, KCH, P).transpose(2, 0, 1))    # [128, NSH, KCH]
        in1.append({"xL": xLm, "W1": W1.astype(np1)})
    r1 = _run_spmd(nc1, in1)
    sup1 = np.concatenate([r1.results[m]["s1"] for m in range(M)], axis=0)

    # window-slot permutation helpers (slot j on core m = window perm[j])
    def unslot(block, m, H):
        """[NWIN*P, H] slot-blocked -> [NSH, H] node-ordered for core m."""
        perm = eshards[m][2]
        out = np.empty((NSH, H), block.dtype)
        for j in range(NWIN):
            wj = int(perm[j])
            r = min(P, NSH - wj * P)
            out[wj * P:wj * P + r] = block[j * P:j * P + r]
        return out

    def toslot(arr, m):
        """[NSH, H] node-ordered -> [NWIN*P, H] slot-blocked for core m."""
        perm = eshards[m][2]
        out = np.zeros((NWIN * P, arr.shape[1]), arr.dtype)
        for j in range(NWIN):
            wj = int(perm[j])
            r = min(P, NSH - wj * P)
            out[j * P:j * P + r] = arr[wj * P:wj * P + r]
        return out

    # ---- L2: h1 + support23 shards
    np2 = NPDT[SUP1_DT]
    sup1 = sup1.astype(np2)
    in2 = [{"sup1A": sup1[:SPLIT], "sup1B": sup1[N - SPLIT:],
            "eidx": eshards[m][0], "emeta": eshards[m][1],
            "W23": W23, "iota": iota.astype(np2)}
           for m in range(M)]
    r2 = _run_spmd(nc2, in2)
    sup23 = np.concatenate(
        [unslot(r2.results[m]["s23"], m, H23) for m in range(M)], axis=0)

    # ---- L3: mu, logvar, z shards
    np3 = NPDT[SUP23_DT]
    sup23 = sup23.astype(np3)
    in3 = [{"sup23A": sup23[:SPLIT], "sup23B": sup23[N - SPLIT:],
            "eidx": eshards[m][0], "emeta": eshards[m][1],
            "iota": iota.astype(np3),
            "epss": toslot(eps[m * NSH:(m + 1) * NSH], m)}
           for m in range(M)]
    r3 = _run_spmd(nc3, in3)
    outs = [unslot(r3.results[m]["out3"], m, 3 * H2) for m in range(M)]
    full = np.concatenate(outs, axis=0)
    z, mu, logvar = full[:, 0:H2], full[:, H2:H23], full[:, H23:3 * H2]
    return (np.ascontiguousarray(z), np.ascontiguousarray(mu),
            np.ascontiguousarray(logvar))
